# revision 22
# baseline (speedup 1.0000x reference)
"""Bass/Trainium2 kernel for DocRE bilinear segment-reduce model.

Shapes (hardcoded): B=4, L=1024, H=768, NH=12, E=24, M=4, P=552, NL=97, BLK=64.
Sharding: 8 cores = (batch b = core//2) x (half of the 552 head-tail pairs).
One SPMD program; all per-core differences flow through input data
(gathered rows + one-hot matrices built on host from the integer indices).
"""

import dataclasses
import numpy as np
import ml_dtypes

import concourse.bass as bass
import concourse.bacc as bacc
import concourse.tile as tile
from concourse import mybir
from concourse.bass_utils import run_bass_kernel_spmd

B, L, H, NH, E, M, P, NL, BLK = 4, 1024, 768, 12, 24, 4, 552, 97, 64
G = H // BLK            # 12 blocks
R = P // 2              # 276 rows per core
EM = E * M              # 96 gathered mentions
HL = NH * L             # 12288
K = H * BLK             # 49152 bilinear contraction
F32 = mybir.dt.float32
BF16 = mybir.dt.bfloat16

_CACHE = {}


def _bcast_src(ap, rep):
    """Source AP [1, F] -> [1, (rep, F)] with step-0 free dim (DMA broadcast)."""
    return dataclasses.replace(ap, ap=[ap.ap[0], [0, rep]] + ap.ap[1:])


def _build_program():
    nc = bacc.Bacc("TRN2", target_bir_lowering=False, debug=False, num_devices=8)
    dp = nc.declare_dram_parameter
    EMG = dp("EMG", [EM, H], F32, isOutput=False)        # gathered mention embeddings
    SUME = dp("SUME", [EM, E], F32, isOutput=False)      # mask one-hot  (logsumexp sum)
    AMG = dp("AMG", [EM, HL], F32, isOutput=False)       # gathered mention attn rows (h-major)
    SUMW = dp("SUMW", [EM, 128], F32, isOutput=False)      # mask/denom/sqrt(NH) one-hot
    OHH = dp("OHH", [128, R], BF16, isOutput=False)         # head-entity one-hot
    OHT = dp("OHT", [128, R], BF16, isOutput=False)         # tail-entity one-hot
    SEQ = dp("SEQ", [L, H], BF16, isOutput=False)         # sequence_output[b]
    WHT = dp("WHT", [2 * H, H], BF16, isOutput=False)     # Wh.T
    WTT = dp("WTT", [2 * H, H], BF16, isOutput=False)     # Wt.T
    WBT = dp("WBT", [K, NL], BF16, isOutput=False)        # Wb.T
    BHS = dp("BHS", [128, 6], F32, isOutput=False)       # bh as [128,6] per o-chunk
    BTS = dp("BTS", [128, 6], F32, isOutput=False)
    BBS = dp("BBS", [NL, 1], F32, isOutput=False)
    SEL = dp("SEL", [64 * 128, 128], BF16, isOutput=False)
    OUT = dp("OUT", [NL, R], F32, isOutput=True)         # logits^T

    with tile.TileContext(nc) as tc:
        with (
            tc.tile_pool(name="persist", bufs=1) as pp,
            tc.tile_pool(name="stream", bufs=6) as sp,
            tc.tile_pool(name="wstream", bufs=3) as wp,
            tc.tile_pool(name="wbt", bufs=6) as wbp,
            tc.tile_pool(name="rep", bufs=6) as repp,
            tc.tile_pool(name="blt", bufs=16) as bltp,
        ):
            # ---- load persistent small inputs
            def load(name, ap, shape, tag, dt=F32):
                t = pp.tile(shape, dt, tag=tag)
                nc.sync.dma_start(t[:], ap)
                return t

            emg = load("EMG", EMG[:], [EM, H], "emg")
            sume = load("SUME", SUME[:], [EM, E], "sume")
            sumw = load("SUMW", SUMW[:], [EM, 128], "sumw")
            ohh = load("OHH", OHH[:], [128, R], "ohh", BF16)
            oht = load("OHT", OHT[:], [128, R], "oht", BF16)
            bhs = load("BHS", BHS[:], [128, 6], "bhs")
            bts = load("BTS", BTS[:], [128, 6], "bts")
            bbs = load("BBS", BBS[:], [NL, 1], "bbs")
            selt = []
            for w in range(8):
                st8 = pp.tile([128, 8 * 128], BF16, tag=f"sel8_{w}", name=f"sel8_{w}")
                nc.gpsimd.dma_start(
                    st8[:].rearrange("p (v c) -> p v c", v=8),
                    SEL[w * 1024:(w + 1) * 1024, :].rearrange("(v p) c -> p v c", p=128))
                selt.extend(st8[:, v * 128:(v + 1) * 128] for v in range(8))
            seqt = [load("SEQ", SEQ[q * 128:(q + 1) * 128, :], [128, H], f"seq{q}", BF16)
                    for q in range(8)]
            ones = pp.tile([128, 128], F32, tag="ones", name="ones")
            nc.vector.memset(ones[:], 1.0)
            onesb = pp.tile([128, 1], BF16, tag="onesb", name="onesb")
            nc.vector.memset(onesb[:], 1.0)

            # ---- phase 1: entity embeddings = ln(sum_m mask * exp(m_emb))
            expt = pp.tile([EM, H], F32, tag="expt", name="expt")
            nc.scalar.activation(expt[:], emg[:], mybir.ActivationFunctionType.Exp)
            eet = pp.tile([E, H], BF16, tag="eet", name="eet")
            with tc.tile_pool(name="ps1", bufs=2, space="PSUM") as ps1:
              for half in range(2):
                pe = ps1.tile([E, 384], F32, tag="ee_ps", name="ee_ps")
                nc.tensor.matmul(pe[:], sume[:], expt[:, half * 384:(half + 1) * 384],
                                 start=True, stop=True)
                nc.scalar.activation(eet[:, half * 384:(half + 1) * 384], pe[:],
                                     mybir.ActivationFunctionType.Ln)

            # ---- phase 2: entity attentions A2 [E, (h,l)] = sum_m (mask/denom/sqrt NH) * att
            a2t = pp.tile([128, HL], BF16, tag="a2t", name="a2t")
            with tc.tile_pool(name="ps2", bufs=3, space="PSUM") as ps2:
              for i2 in range(HL // 1024):
                amg_c = sp.tile([EM, 1024], F32, tag="amg", name="amg")
                nc.scalar.dma_start(amg_c[:], AMG[:, i2 * 1024:(i2 + 1) * 1024])
                for half2 in range(2):
                    i = i2 * 2 + half2
                    pa = ps2.tile([128, 512], F32, tag="a2_ps", name="a2_ps")
                    nc.tensor.matmul(pa[:], sumw[:],
                                     amg_c[:, half2 * 512:(half2 + 1) * 512],
                                     start=True, stop=True)
                    if i % 2 == 0:
                        nc.vector.tensor_copy(a2t[:, i * 512:(i + 1) * 512], pa[:])
                    else:
                        nc.scalar.copy(a2t[:, i * 512:(i + 1) * 512], pa[:])

            # ---- phase 3: hs^T / ts^T gathers  [128d, R] x 6
            hst, tst = [], []
            with tc.tile_pool(name="ps3", bufs=3, space="PSUM") as ps3:
              for oc in range(6):
                for (oh, dst_list, tag) in ((ohh, hst, "hs"), (oht, tst, "ts")):
                    pg = ps3.tile([128, R], F32, tag="gat_ps", name="gat_ps")
                    nc.tensor.matmul(pg[:], eet[:, oc * 128:(oc + 1) * 128], oh[0:E, :],
                                     start=True, stop=True)
                    t = pp.tile([128, R], BF16, tag=f"{tag}{oc}", name=f"{tag}{oc}")
                    nc.scalar.copy(t[:], pg[:])
                    dst_list.append(t)

            # ---- phase 4: ht_att (pre-normalization) per l-chunk
            htacc = []
            with tc.tile_pool(name="ps4", bufs=2, space="PSUM") as ps4:
              for q in range(8):
                acc = pp.tile([128, R], BF16, tag=f"ht{q}", name=f"ht{q}")
                w4 = sp.tile([128, NH, R], F32, tag="w4", name="w4", bufs=2)
                for hp in range(NH // 2):
                    # two heads share one 2-bank PSUM tile -> one mul, one evac.
                    # The 4 gathers run in 4 PE row-groups concurrently (K=24).
                    hh2 = ps4.tile([128, 1024], F32, tag="hh_ps", name="hh_ps", bufs=2)
                    tt2 = ps4.tile([128, 1024], F32, tag="tt_ps", name="tt_ps", bufs=2)
                    for kk in range(2):
                        h = hp * 2 + kk
                        lsl = slice(h * L + q * 128, h * L + (q + 1) * 128)
                        rg_h, rg_t = kk * 64, kk * 64 + 32
                        nc.tensor.matmul(hh2[:, kk * 512:kk * 512 + R],
                                         a2t[rg_h:rg_h + E, lsl],
                                         ohh[rg_h:rg_h + E, :],
                                         start=True, stop=True,
                                         tile_position=(rg_h, 0))
                        nc.tensor.matmul(tt2[:, kk * 512:kk * 512 + R],
                                         a2t[rg_t:rg_t + E, lsl],
                                         oht[rg_t:rg_t + E, :],
                                         start=True, stop=True,
                                         tile_position=(rg_t, 0))
                    # DVE can read at most one PSUM operand: evac T via ScalarE
                    tview = dataclasses.replace(
                        tt2[:], ap=[tt2[:].ap[0], [512, 2], [1, R]])
                    hview = dataclasses.replace(
                        hh2[:], ap=[hh2[:].ap[0], [512, 2], [1, R]])
                    tsb = sp.tile([128, 2, R], F32, tag="tsb", name="tsb")
                    nc.scalar.copy(tsb[:], tview)
                    nc.vector.tensor_tensor(w4[:, hp * 2:hp * 2 + 2, :], hview,
                                            tsb[:], mybir.AluOpType.mult)
                # tree-sum over the 12 heads; 2 adds on GpSimd (SBUF-only ok)
                nc.vector.tensor_add(w4[:, 0:6, :], w4[:, 0:6, :], w4[:, 6:12, :])
                nc.gpsimd.tensor_add(w4[:, 0:3, :], w4[:, 0:3, :], w4[:, 3:6, :])
                nc.vector.tensor_add(w4[:, 0, :], w4[:, 0, :], w4[:, 1, :])
                nc.gpsimd.tensor_add(acc[:], w4[:, 0, :], w4[:, 2, :])
                htacc.append(acc)

            # ---- phase 5: 1/(sum_l ht + 1e-5), broadcast to 128 partitions
            invd = pp.tile([128, R], F32, tag="invd", name="invd")
            with tc.tile_pool(name="ps5", bufs=1, space="PSUM") as ps5:
                psum_s = ps5.tile([1, R], F32, tag="s_ps", name="s_ps")
                for q in range(8):
                    nc.tensor.matmul(psum_s[:], onesb[:], htacc[q][:],
                                     start=(q == 0), stop=(q == 7))
                invd1 = pp.tile([1, R], F32, tag="invd1", name="invd1")
                nc.vector.tensor_scalar_add(invd1[:], psum_s[:], 1e-5)
                nc.vector.reciprocal(invd1[:], invd1[:])
                pb = ps5.tile([128, R], F32, tag="invd_ps", name="invd_ps")
                nc.tensor.matmul(pb[:], ones[0:1, :], invd1[:], start=True, stop=True)
                nc.scalar.copy(invd[:], pb[:])

            # ---- phase 6: rs^T chunks (normalization folded into evac)
            rst = []
            with tc.tile_pool(name="ps6", bufs=2, space="PSUM") as ps6:
              for dc in range(6):
                pr = ps6.tile([128, R], F32, tag="rs_ps", name="rs_ps")
                for q in range(8):
                    nc.tensor.matmul(pr[:], seqt[q][:, dc * 128:(dc + 1) * 128],
                                     htacc[q][:], start=(q == 0), stop=(q == 7))
                t = pp.tile([128, R], BF16, tag=f"rs{dc}", name=f"rs{dc}")
                nc.vector.tensor_mul(t[:], pr[:], invd[:])
                rst.append(t)

            # ---- phase 7: projections zh^T = tanh(Wh^T @ [hs; rs] + bh), same for zt
            zht, ztt = [], []
            for (wdram, inv, bias, out_list, tag) in (
                    (WHT, hst, bhs, zht, "zh"), (WTT, tst, bts, ztt, "zt")):
              with tc.tile_pool(name=f"ps7{tag}", bufs=1, space="PSUM") as ps7:
                pps = [ps7.tile([128, R], F32, tag=f"{tag}_ps{oc}", name=f"{tag}_ps{oc}") for oc in range(6)]
                for k2 in range(6):
                    wt2 = wp.tile([128, 2, H], BF16, tag="wproj", name="wproj")
                    nc.sync.dma_start(
                        wt2[:],
                        wdram[k2 * 256:(k2 + 1) * 256, :].rearrange("(j p) n -> p j n", p=128))
                    for kk in range(2):
                        k = k2 * 2 + kk
                        rhs = inv[k] if k < 6 else rst[k - 6]
                        for oc in range(6):
                            nc.tensor.matmul(pps[oc][:],
                                             wt2[:, kk, oc * 128:(oc + 1) * 128],
                                             rhs[:], start=(k == 0), stop=(k == 11))
                for oc in range(6):
                    t = pp.tile([128, R], BF16, tag=f"{tag}{oc}", name=f"{tag}{oc}")
                    nc.scalar.activation(t[:], pps[oc][:],
                                         mybir.ActivationFunctionType.Tanh,
                                         bias=bias[:, oc:oc + 1])
                    out_list.append(t)

            # ---- phase 8: zt replicated per group g: [zt_g; zt_g]
            ztr = []
            for g in range(G):
                src = ztt[g // 2][(g % 2) * 64:(g % 2) * 64 + 64, :]
                t = pp.tile([128, R], BF16, tag=f"ztr{g}", name=f"ztr{g}")
                nc.sync.dma_start(t[0:64, :], src)
                nc.sync.dma_start(t[64:128, :], src)
                ztr.append(t)

            # ---- phase 9: bilinear logits^T = sum_k WbT[k,:]^T * bl^T[k,:]
            ps9_cm = tc.tile_pool(name="ps9", bufs=1, space="PSUM")
            ps9 = ps9_cm.__enter__()
            ps9r_cm = tc.tile_pool(name="ps9r", bufs=6, space="PSUM")
            ps9r = ps9r_cm.__enter__()
            lt = ps9.tile([NL, R], F32, tag="lt_ps", name="lt_ps")
            nchunks = G * 32
            ci = 0
            for g in range(G):
                zsrc = zht[g // 2]
                for i0 in range(0, 64, 2):
                    sv = selt[(g % 2) * 32 + i0 // 2]
                    rep = ps9r.tile([128, R], F32, tag="rep_ps", name="rep_ps")
                    nc.tensor.matmul(rep[:], sv[:], zsrc[:],
                                     start=True, stop=True)
                    blt = bltp.tile([128, R], BF16, tag="blt", name="blt")
                    if ci % 3 == 2:
                        # keep DVE free: evac via ACT, multiply on GpSimd
                        rsb = repp.tile([128, R], BF16, tag="rsb", name="rsb")
                        nc.scalar.copy(rsb[:], rep[:])
                        nc.gpsimd.tensor_mul(blt[:], rsb[:], ztr[g][:])
                    else:
                        nc.vector.tensor_mul(blt[:], rep[:], ztr[g][:])
                    if ci % 8 == 0:
                        wbc8 = wbp.tile([128, 8, NL], BF16, tag="wbc", name="wbc")
                        kbase = g * 4096 + i0 * 64
                        qeng = nc.sync if (ci // 8) % 2 == 0 else nc.scalar
                        qeng.dma_start(
                            wbc8[:],
                            WBT[kbase:kbase + 1024, :].rearrange("(j p) n -> p j n", p=128))
                    nc.tensor.matmul(lt[:], wbc8[:, ci % 8, :], blt[:],
                                     start=(ci == 0), stop=(ci == nchunks - 1))
                    ci += 1

            lout = pp.tile([NL, R], F32, tag="lout", name="lout")
            nc.vector.tensor_scalar_add(lout[:], lt[:], bbs[:, 0:1])
            nc.sync.dma_start(OUT[:], lout[:])
            ps9r_cm.__exit__(None, None, None)
            ps9_cm.__exit__(None, None, None)

    nc.finalize()
    return nc


def _sel_variants():
    sel = np.zeros((64, 128, 128), np.float32)
    for v in range(64):
        half, i0 = v // 32, (v % 32) * 2
        for p in range(128):
            sel[v, half * 64 + i0 + p // 64, p] = 1.0
    return sel.reshape(64 * 128, 128).astype(ml_dtypes.bfloat16)


def _prep_core_inputs(c, sequence_output, attention, mention_mask, Wh, bh, Wt, bt,
                      Wb, bb, mention_idx, hts):
    b, half = c // 2, c % 2
    seq_b = np.ascontiguousarray(sequence_output[b])              # [L, H]
    idx = mention_idx[b].astype(np.int64).reshape(EM)             # [96]
    mask = mention_mask[b].astype(np.float32)                     # [E, M]
    denom = mask.sum(-1)                                          # [E]

    emg = np.ascontiguousarray(seq_b[idx])                        # [96, H]
    amg = np.ascontiguousarray(
        attention[b][:, idx, :].transpose(1, 0, 2).reshape(EM, HL))

    sume = np.zeros((EM, E), np.float32)
    sumw = np.zeros((EM, 128), np.float32)
    s = 1.0 / np.sqrt(np.float32(NH))
    for e in range(E):
        for m in range(M):
            sume[e * M + m, e] = mask[e, m]
            for rg in range(4):
                sumw[e * M + m, rg * 32 + e] = mask[e, m] / denom[e] * s

    hts_c = hts[b, half * R:(half + 1) * R].astype(np.int64)      # [R, 2]
    ohh = np.zeros((128, R), np.float32)
    oht = np.zeros((128, R), np.float32)
    for rg in range(4):
        ohh[rg * 32 + hts_c[:, 0], np.arange(R)] = 1.0
        oht[rg * 32 + hts_c[:, 1], np.arange(R)] = 1.0

    return {
        "EMG": emg, "SUME": sume, "AMG": amg, "SUMW": sumw,
        "OHH": ohh.astype(ml_dtypes.bfloat16), "OHT": oht.astype(ml_dtypes.bfloat16), "SEQ": seq_b.astype(ml_dtypes.bfloat16),
        "WHT": np.ascontiguousarray(Wh.T).astype(ml_dtypes.bfloat16), "WTT": np.ascontiguousarray(Wt.T).astype(ml_dtypes.bfloat16),
        "WBT": np.ascontiguousarray(Wb.T).astype(ml_dtypes.bfloat16),
        "BHS": np.ascontiguousarray(bh.reshape(6, 128).T),
        "BTS": np.ascontiguousarray(bt.reshape(6, 128).T),
        "BBS": bb.reshape(NL, 1).astype(np.float32),
        "SEL": _sel_variants(),
    }


def kernel(sequence_output, attention, mention_mask, Wh, bh, Wt, bt, Wb, bb,
           mention_idx, hts):
    if "nc" not in _CACHE:
        _CACHE["nc"] = _build_program()
    nc = _CACHE["nc"]

    args = (np.asarray(sequence_output, np.float32), np.asarray(attention, np.float32),
            np.asarray(mention_mask, np.float32), np.asarray(Wh, np.float32),
            np.asarray(bh, np.float32), np.asarray(Wt, np.float32),
            np.asarray(bt, np.float32), np.asarray(Wb, np.float32),
            np.asarray(bb, np.float32), np.asarray(mention_idx),
            np.asarray(hts))
    in_maps = [_prep_core_inputs(c, *args) for c in range(8)]
    res = run_bass_kernel_spmd(nc, in_maps, list(range(8))).results

    out = np.empty((B, P, NL), np.float32)
    for c in range(8):
        b, half = c // 2, c % 2
        out[b, half * R:(half + 1) * R, :] = np.asarray(res[c]["OUT"]).T
    return out
